# revision 22
# baseline (speedup 1.0000x reference)
import numpy as np, sys, os, math, functools
sys.path.insert(0, "/opt/trn_rl_repo")

V, D, L = 32000, 1024, 8
HQ, HKV, HD = 16, 4, 64
H = 2752
B, S = 2, 1024
WINDOW, GEVERY = 256, 4
EPS, BASE = 1e-6, 10000.0
NCORES = 8
T = 256
VSH = V // NCORES
NEG = -30000.0
SCALE = 1.0 / 8.0
HCHUNKS = [(i * 128, 128) for i in range(21)] + [(2688, 64)]
# q-head slot order: slot s=2f+j holds original head QPERM[s]; kvh(QPERM[s])%2==j
QPERM = [0, 4, 1, 5, 2, 6, 3, 7, 8, 12, 9, 13, 10, 14, 11, 15]
KVH = [h // 4 for h in QPERM]
# uniform attention schedule (same instruction stream on every core):
#   t=0 (low query block, b=cp in 0..3):  key blocks 0..3
#   t=1 (high query block, b=7-cp in 4..7): global: kb 0..7; local: kb 2..7
SCHED = {  # (is_global, t) -> list of key blocks
    (True, 0): list(range(4)), (True, 1): list(range(8)),
    (False, 0): list(range(4)), (False, 1): list(range(2, 8)),
}

def _core_blocks(c):
    cp = c % 4
    return cp, 7 - cp

def _inv_freq():
    return 1.0 / (BASE ** (np.arange(0, HD, 2, dtype=np.float64) / HD))

def _mask_group(b, kbs, local):
    """Additive mask [128, len(kbs)*128] in S^T coords (rows=key, cols=query)."""
    dk = np.arange(128)[:, None]
    dq = np.arange(128)[None, :]
    cols = []
    for kb in kbs:
        k = kb * 128 + dk          # [128,1] absolute key pos
        q = b * 128 + dq           # [1,128] absolute query pos
        valid = k <= q
        if local:
            valid = valid & ((q - k) < WINDOW)
        cols.append(np.where(valid, 0.0, NEG))
    return np.concatenate(cols, axis=1).astype(np.float16)

def _host_prep(idx, emb, Wq, Wk, Wv, Wo, w1, w3, w2, n1, n2, nf):
    idx = np.asarray(idx); emb = np.asarray(emb, dtype=np.float32)
    Wq = np.asarray(Wq, dtype=np.float32); Wo = np.asarray(Wo, dtype=np.float32)
    n1 = np.asarray(n1, dtype=np.float32); n2 = np.asarray(n2, dtype=np.float32)
    nf = np.asarray(nf, dtype=np.float32)
    invf = _inv_freq()
    wqg = Wq * n1[:, :, None]
    wq2 = np.ascontiguousarray(
        np.concatenate([wqg[:, :, h*64:(h+1)*64] for h in QPERM], axis=2)
    ).astype(np.float16)
    wo2 = np.ascontiguousarray(
        np.concatenate([Wo[:, h*64:(h+1)*64, :] for h in QPERM], axis=1)
    ).astype(np.float16)
    wkh = (np.asarray(Wk, np.float32) * n1[:, :, None]).astype(np.float16)
    wvh = (np.asarray(Wv, np.float32) * n1[:, :, None]).astype(np.float16)
    w1h = (np.asarray(w1, np.float32) * n2[:, :, None]).astype(np.float16)
    w3h = (np.asarray(w3, np.float32) * n2[:, :, None]).astype(np.float16)
    w2h = np.asarray(w2, dtype=np.float16)
    embT = np.ascontiguousarray((emb.T * nf[:, None]).astype(np.float16))
    in_maps = []
    for c in range(NCORES):
        s = c // 4
        bA, bB = _core_blocks(c)
        tok = np.concatenate([idx[s, bA*128:(bA+1)*128], idx[s, bB*128:(bB+1)*128]])
        x0T = np.ascontiguousarray(emb[tok].T.astype(np.float16))
        pos = np.concatenate([bA*128 + np.arange(128), bB*128 + np.arange(128)])
        ang = pos[:, None].astype(np.float64) * invf[None, :]
        cosq = np.cos(ang).astype(np.float32).reshape(2, 128, 32)
        sinq = np.sin(ang).astype(np.float32).reshape(2, 128, 32)
        in_maps.append({
            "x0T": x0T,
            "wq": wq2, "wk": wkh, "wv": wvh, "wo": wo2,
            "w1": w1h, "w3": w3h, "w2": w2h,
            "cosq": cosq, "sinq": sinq,
            "mg0": _mask_group(bA, SCHED[(True, 0)], False),
            "mg1": _mask_group(bB, SCHED[(True, 1)], False),
            "ml0": _mask_group(bA, SCHED[(False, 0)], True),
            "ml1": _mask_group(bB, SCHED[(False, 1)], True),
            "embT": embT[:, c*VSH:(c+1)*VSH],
        })
    return in_maps

def _unperm_rows():
    perm = np.zeros(2048, dtype=np.int64)
    for r in range(2048):
        rr, rem = divmod(r, 256)
        slot, p = divmod(rem, 128)
        samp = rr // 4
        bA, bB = _core_blocks(rr)
        blk = bA if slot == 0 else bB
        perm[r] = samp * S + blk * 128 + p
    inv = np.zeros(2048, dtype=np.int64)
    inv[perm] = np.arange(2048)
    return inv

def _assemble(outs):
    full = np.empty((2048, V), dtype=np.float32)
    for c in range(NCORES):
        full[:, c*VSH:(c+1)*VSH] = np.asarray(outs[c]["logits"], dtype=np.float32)
    inv = _unperm_rows()
    return full[inv].reshape(B, S, V)

def _build_nc():
    import concourse.bass as bass
    import concourse.bacc as bacc
    import concourse.mybir as mybir
    from concourse.tile import TileContext
    from concourse.masks import make_identity
    F32, F16 = mybir.dt.float32, mybir.dt.float16
    AF = mybir.ActivationFunctionType

    nc = bacc.Bacc("TRN2", target_bir_lowering=False, debug=False, num_devices=NCORES)
    P = {}
    def inp(name, shape, dt=F16):
        P[name] = nc.declare_dram_parameter(name, list(shape), dt, isOutput=False)
    inp("x0T", (D, T), F16)
    inp("wq", (L, D, D)); inp("wk", (L, D, 256)); inp("wv", (L, D, 256))
    inp("wo", (L, D, D))
    inp("w1", (L, D, H)); inp("w3", (L, D, H)); inp("w2", (L, H, D))
    inp("cosq", (2, 128, 32), F32); inp("sinq", (2, 128, 32), F32)
    inp("mg0", (128, 512)); inp("mg1", (128, 1024))
    inp("ml0", (128, 512)); inp("ml1", (128, 768))
    inp("embT", (D, VSH))
    logits = nc.declare_dram_parameter("logits", [2048, VSH], F16, isOutput=True)

    kv_in  = [nc.dram_tensor(f"kv_in{l}",  [4, 128, 256], F16) for l in range(L)]
    kv_out = [nc.dram_tensor(f"kv_out{l}", [16, 128, 256], F16) for l in range(L)]
    xf_in  = nc.dram_tensor("xf_in", [D, T], F16)
    xf_out = nc.dram_tensor("xf_out", [NCORES * D, T], F16, addr_space="Shared")
    RG_KV = [[0, 1, 2, 3], [4, 5, 6, 7]]
    RG_ALL = [list(range(NCORES))]

    with TileContext(nc) as tc:
      with tc.tile_pool(name="pers", bufs=1) as pers, \
           tc.tile_pool(name="wpool", bufs=2) as wp, \
           tc.tile_pool(name="act", bufs=2) as act, \
           tc.tile_pool(name="attn", bufs=2) as atp, \
           tc.tile_pool(name="small", bufs=2) as sm, \
           tc.tile_pool(name="ppb", bufs=2, space="PSUM") as ppb, \
           tc.tile_pool(name="ppm", bufs=3, space="PSUM") as ppm, \
           tc.tile_pool(name="ppav", bufs=2, space="PSUM") as ppav, \
           tc.tile_pool(name="ppt", bufs=1, space="PSUM") as ppt:

        dma = nc.sync.dma_start
        gdma = nc.gpsimd.dma_start
        xT = [pers.tile([128, T], F16, tag=f"xT{d}", name=f"xT{d}") for d in range(8)]
        for d in range(8):
            dma(out=xT[d], in_=P["x0T"][d*128:(d+1)*128, :])
        cosA = pers.tile([128, 32], F32, tag="cosA", name="cosA"); dma(out=cosA, in_=P["cosq"][0])
        cosB = pers.tile([128, 32], F32, tag="cosB", name="cosB"); dma(out=cosB, in_=P["cosq"][1])
        sinA = pers.tile([128, 32], F32, tag="sinA", name="sinA"); dma(out=sinA, in_=P["sinq"][0])
        sinB = pers.tile([128, 32], F32, tag="sinB", name="sinB"); dma(out=sinB, in_=P["sinq"][1])
        mg = {}
        for nm, w in [("mg0", 512), ("mg1", 1024), ("ml0", 512), ("ml1", 768)]:
            mg[nm] = pers.tile([128, w], F16, tag=nm, name=nm)
            dma(out=mg[nm], in_=P[nm][:, :])
        idn = pers.tile([128, 128], F16, tag="idn", name="idn")
        make_identity(nc, idn)
        ones16 = pers.tile([128, 1], F16, tag="ones16", name="ones16")
        nc.vector.memset(ones16, 1.0)
        epst = pers.tile([1, 1], F32, tag="epst", name="epst")
        nc.vector.memset(epst, EPS)
        epst128 = pers.tile([128, 1], F32, tag="epst128", name="epst128")
        nc.vector.memset(epst128, EPS)
        kT_par = [[pers.tile([128, 1024], F16, tag=f"kT{p}_{i}", name=f"kT{p}_{i}")
                   for i in range(2)] for p in range(2)]
        v_par = [pers.tile([128, 8, 4, 65], F16, tag=f"vf{p}", name=f"vf{p}")
                 for p in range(2)]
        for p in range(2):
            nc.vector.memset(v_par[p][:, :, :, 64:65], 1.0)

        def rmsnorm(outdt=F16):
            ss = ppm.tile([1, T], F32, tag="pm", name="ss")
            for d in range(8):
                x2 = act.tile([128, T], F16, tag="x2", name="x2")
                nc.vector.tensor_mul(out=x2, in0=xT[d], in1=xT[d])
                nc.tensor.matmul(ss, lhsT=ones16, rhs=x2, start=(d == 0), stop=(d == 7))
            rrow = sm.tile([1, T], F32, tag="rrow", name="rrow")
            nc.scalar.activation(out=rrow, in_=ss, func=AF.Sqrt, scale=1.0/D, bias=epst[0:1, 0:1])
            rrec = sm.tile([1, T], F32, tag="rrec", name="rrec")
            nc.vector.reciprocal(out=rrec, in_=rrow)
            rb = act.tile([128, T], F32, tag="rb", name="rb")
            nc.gpsimd.partition_broadcast(rb[:], rrec[:])
            out = []
            for d in range(8):
                h = act.tile([128, T], outdt, tag=f"hT{d}", name=f"hT{d}", bufs=1)
                nc.vector.tensor_mul(out=h, in0=xT[d], in1=rb)
                out.append(h)
            return out

        def rope_tok(ps, cost, sint, outt, nheads):
            ev = ps.rearrange("p (h f two) -> p h f two", two=2, f=32)
            ov = outt.rearrange("p (h f two) -> p h f two", two=2, f=32)
            cb = cost[:].rearrange("p (o f) -> p o f", o=1).to_broadcast((128, nheads, 32))
            sb = sint[:].rearrange("p (o f) -> p o f", o=1).to_broadcast((128, nheads, 32))
            t1 = sm.tile([128, nheads, 32], F32, tag="ropet1", name="ropet1")
            t2 = sm.tile([128, nheads, 32], F32, tag="ropet2", name="ropet2")
            nc.vector.tensor_mul(out=t1, in0=ev[:, :, :, 0], in1=cb)
            nc.vector.tensor_mul(out=t2, in0=ev[:, :, :, 1], in1=sb)
            nc.vector.tensor_sub(out=ov[:, :, :, 0], in0=t1, in1=t2)
            nc.vector.tensor_mul(out=t1, in0=ev[:, :, :, 0], in1=sb)
            nc.vector.tensor_mul(out=t2, in0=ev[:, :, :, 1], in1=cb)
            nc.vector.tensor_add(out=ov[:, :, :, 1], in0=t1, in1=t2)

        for l in range(L):
            is_global = ((l + 1) % GEVERY) == 0
            # token-major inv-rms stats; normalization folded into rope cos/sin
            # (q and k both carry rrec -> scores get rrec_q*rrec_k) and V evict
            ssT = [ppm.tile([128, 1], F32, tag="pm", name=f"ssT{t}") for t in range(2)]
            for d in range(8):
                x2 = act.tile([128, T], F16, tag="x2", name="x2")
                nc.vector.tensor_mul(out=x2, in0=xT[d], in1=xT[d])
                for t2_ in range(2):
                    nc.tensor.matmul(ssT[t2_], lhsT=x2[:, t2_*128:(t2_+1)*128],
                                     rhs=ones16, start=(d == 0), stop=(d == 7))
            srt = sm.tile([128, 2], F32, tag="srt", name="srt")
            for t2_ in range(2):
                nc.scalar.activation(out=srt[:, t2_:t2_+1], in_=ssT[t2_], func=AF.Sqrt,
                                     scale=1.0/D, bias=epst128[:, 0:1])
            rrT = sm.tile([128, 2], F32, tag="rrT", name="rrT")
            nc.vector.reciprocal(out=rrT, in_=srt)
            csS = []
            for t2_, (ct, st_) in enumerate([(cosA, sinA), (cosB, sinB)]):
                cS = sm.tile([128, 32], F32, tag=f"cS{t2_}", name=f"cS{t2_}")
                sS = sm.tile([128, 32], F32, tag=f"sS{t2_}", name=f"sS{t2_}")
                nc.vector.tensor_scalar_mul(out=cS, in0=ct, scalar1=rrT[:, t2_:t2_+1])
                nc.vector.tensor_scalar_mul(out=sS, in0=st_, scalar1=rrT[:, t2_:t2_+1])
                csS.append((cS, sS))
            # ---- K, V projections (token-major) ----
            wkv = wp.tile([128, 8, 512], F16, tag="wkv", name="wkv")
            dma(out=wkv[:, :, 0:256], in_=P["wk"][l].rearrange("(dc p) f -> p dc f", p=128))
            dma(out=wkv[:, :, 256:512], in_=P["wv"][l].rearrange("(dc p) f -> p dc f", p=128))
            ktok, vtok = [], []
            for t2_ in range(2):
                pskv = ppb.tile([128, 512], F32, tag="pb", name="pskv")
                for d in range(8):
                    nc.tensor.matmul(pskv, lhsT=xT[d][:, t2_*128:(t2_+1)*128], rhs=wkv[:, d, :],
                                     start=(d == 0), stop=(d == 7))
                kt = atp.tile([128, 256], F16, tag=f"ktok{t2_}", name=f"ktok{t2_}")
                rope_tok(pskv[:, 0:256], csS[t2_][0], csS[t2_][1], kt, 4)
                ktok.append(kt)
                vt = atp.tile([128, 256], F16, tag=f"vtok{t2_}", name=f"vtok{t2_}")
                nc.vector.tensor_scalar_mul(out=vt, in0=pskv[:, 256:512],
                                            scalar1=rrT[:, t2_:t2_+1])
                vtok.append(vt)
            kT_sb = [atp.tile([128, 256], F16, tag=f"kTsb{i}", name=f"kTsb{i}") for i in range(2)]
            for i in range(2):
                for t2_ in range(2):
                    pst = ppt.tile([128, 128], F16, tag="tr", name="pstr")
                    nc.tensor.transpose(pst, ktok[t2_][:, i*128:(i+1)*128], idn)
                    nc.vector.tensor_copy(out=kT_sb[i][:, t2_*128:(t2_+1)*128], in_=pst)
            for i in range(2):
                gdma(out=kv_in[l][i], in_=kT_sb[i])
                gdma(out=kv_in[l][2 + i], in_=vtok[i])
            nc.gpsimd.collective_compute(
                "AllGather", mybir.AluOpType.bypass, replica_groups=RG_KV,
                ins=[kv_in[l].ap()], outs=[kv_out[l].ap()])
            # ---- Q projection + rope + transpose ----
            qtoks = [act.tile([128, D], F16, tag=f"qtok{t}", name=f"qtok{t}", bufs=1) for t in range(2)]
            for hf in range(2):
                wqh = wp.tile([128, 8, 512], F16, tag="wqh", name="wqh")
                dma(out=wqh, in_=P["wq"][l, :, hf*512:(hf+1)*512].rearrange("(dc p) f -> p dc f", p=128))
                for t2_ in range(2):
                    psq = ppb.tile([128, 512], F32, tag="pb", name="psq")
                    for d in range(8):
                        nc.tensor.matmul(psq, lhsT=xT[d][:, t2_*128:(t2_+1)*128],
                                         rhs=wqh[:, d, :],
                                         start=(d == 0), stop=(d == 7))
                    rope_tok(psq, csS[t2_][0], csS[t2_][1],
                             qtoks[t2_][:, hf*512:(hf+1)*512], 8)
            qT = [atp.tile([128, 256], F16, tag=f"qT{f}", name=f"qT{f}", bufs=1) for f in range(8)]
            for t2_ in range(2):
                for f in range(8):
                    pst = ppt.tile([128, 128], F16, tag="tr", name="pstr")
                    nc.tensor.transpose(pst, qtoks[t2_][:, f*128:(f+1)*128], idn)
                    nc.vector.tensor_copy(out=qT[f][:, t2_*128:(t2_+1)*128], in_=pst)
            # ---- assemble gathered K/V (parity buffers) ----
            kT_full = kT_par[l % 2]
            v_full = v_par[l % 2]
            for i in range(2):
                issue = nc.scalar.dma_start if i == 0 else dma
                for b in range(8):
                    r, sl = (b, 0) if b < 4 else (7 - b, 1)
                    issue(out=kT_full[i][:, b*128:(b+1)*128],
                          in_=kv_out[l][r*4 + i, :, sl*128:(sl+1)*128])
            for b in range(8):
                r, sl = (b, 0) if b < 4 else (7 - b, 1)
                issue = gdma if b % 2 == 0 else nc.scalar.dma_start
                issue(out=v_full[:, b, :, 0:64],
                      in_=kv_out[l][r*4 + 2 + sl].rearrange("p (h f) -> p h f", f=64))
            # ---- attention: S^T scores + masked exp + token-major AV ----
            oT = [act.tile([128, 256], F16, tag=f"oT{f}", name=f"oT{f}", bufs=1) for f in range(8)]
            for t in range(2):
                kbs = SCHED[(is_global, t)]
                nb = len(kbs)
                mgt = mg[("mg" if is_global else "ml") + str(t)]
                groups = [(g0, min(4, nb - g0)) for g0 in range(0, nb, 4)]
                for f in range(8):
                    oTok2 = sm.tile([128, 128], F16, tag="oTok", name="oTok")
                    for j in range(2):
                        s = 2 * f + j
                        kvh = KVH[s]
                        kT = kT_full[s // 8]
                        lhq = qT[f][j*64:(j+1)*64, t*128:(t+1)*128]
                        probs = atp.tile([128, 1024], F16, tag="probs", name="probs", bufs=3)
                        for (g0, gw) in groups:
                            st = ppb.tile([128, 512], F32, tag="pb", name="st")
                            clean = is_global and t == 1 and g0 == 0
                            if not clean:
                                nc.tensor.matmul(st[:, 0:gw*128], lhsT=idn,
                                                 rhs=mgt[:, g0*128:(g0+gw)*128],
                                                 start=True, stop=False, skip_group_check=True)
                            for gi in range(gw):
                                kb = kbs[g0 + gi]
                                nc.tensor.matmul(st[:, gi*128:(gi+1)*128],
                                                 lhsT=kT[j*64:(j+1)*64, kb*128:(kb+1)*128],
                                                 rhs=lhq,
                                                 start=clean, stop=(gi == gw - 1),
                                                 skip_group_check=True)
                            nc.scalar.activation(out=probs[:, g0*128:(g0+gw)*128],
                                                 in_=st[:, 0:gw*128], func=AF.Exp, scale=SCALE)
                        av = ppav.tile([128, 65], F32, tag="av", name="av")
                        for bi, kb in enumerate(kbs):
                            nc.tensor.matmul(av, lhsT=probs[:, bi*128:(bi+1)*128],
                                             rhs=v_full[:, kb, kvh, :],
                                             start=(bi == 0), stop=(bi == nb - 1))
                        rec = sm.tile([128, 1], F32, tag="rec", name="rec")
                        nc.vector.reciprocal(out=rec, in_=av[:, 64:65])
                        nc.vector.tensor_scalar_mul(out=oTok2[:, j*64:(j+1)*64],
                                                    in0=av[:, 0:64], scalar1=rec)
                    pst = ppt.tile([128, 128], F16, tag="tr", name="pstr")
                    nc.tensor.transpose(pst, oTok2, idn)
                    nc.vector.tensor_copy(out=oT[f][:, t*128:(t+1)*128], in_=pst)
            # ---- O projection + residual ----
            for d in range(8):
                wod = wp.tile([128, 8, 128], F16, tag="wod", name="wod")
                dma(out=wod, in_=P["wo"][l, :, d*128:(d+1)*128].rearrange("(ft p) c -> p ft c", p=128))
                pso = ppm.tile([128, 256], F32, tag="pm", name="pso")
                for ft in range(8):
                    nc.tensor.matmul(pso, lhsT=wod[:, ft, :], rhs=oT[ft],
                                     start=(ft == 0), stop=(ft == 7))
                nc.vector.tensor_add(out=xT[d], in0=xT[d], in1=pso)
            # ---- FFN ----
            h2 = rmsnorm()
            yT = []
            for c0 in range(0, H, 256):
                cw = min(256, H - c0)
                w1c = wp.tile([128, 8, 256], F16, tag="w1c", name="w1c")
                w3c = wp.tile([128, 8, 256], F16, tag="w3c", name="w3c")
                dma(out=w1c[:, :, 0:cw], in_=P["w1"][l, :, c0:c0+cw].rearrange("(dc p) h -> p dc h", p=128))
                dma(out=w3c[:, :, 0:cw], in_=P["w3"][l, :, c0:c0+cw].rearrange("(dc p) h -> p dc h", p=128))
                for h0 in range(c0, c0 + cw, 128):
                    hw = min(128, c0 + cw - h0)
                    o0 = h0 - c0
                    psu = ppm.tile([128, 256], F32, tag="pm", name="psu")
                    psg = ppm.tile([128, 256], F32, tag="pm", name="psg")
                    for d in range(8):
                        nc.tensor.matmul(psu[0:hw, :], lhsT=w1c[:, d, o0:o0+hw], rhs=h2[d],
                                         start=(d == 0), stop=(d == 7))
                    for d in range(8):
                        nc.tensor.matmul(psg[0:hw, :], lhsT=w3c[:, d, o0:o0+hw], rhs=h2[d],
                                         start=(d == 0), stop=(d == 7))
                    su = act.tile([128, 256], F32, tag="su", name="su")
                    nc.scalar.activation(out=su[0:hw, :], in_=psu[0:hw, :], func=AF.Silu)
                    y = act.tile([128, 256], F16, tag=f"yT{h0}", name=f"yT{h0}", bufs=1)
                    nc.vector.tensor_mul(out=y[0:hw, :], in0=su[0:hw, :], in1=psg[0:hw, :])
                    yT.append(y)
            w2xt = wp.tile([64, D], F16, tag="w2xt", name="w2xt")
            dma(out=w2xt, in_=P["w2"][l, 2688:2752, :])
            nhc = len(HCHUNKS)
            for d in range(8):
                w2d = wp.tile([128, 21, 128], F16, tag="w2d", name="w2d")
                dma(out=w2d, in_=P["w2"][l, 0:2688, d*128:(d+1)*128].rearrange("(hc p) c -> p hc c", p=128))
                ps2 = ppm.tile([128, 256], F32, tag="pm", name="ps2")
                for ci, (h0, hw) in enumerate(HCHUNKS):
                    lh = w2d[:, h0 // 128, :] if hw == 128 else w2xt[:, d*128:(d+1)*128]
                    nc.tensor.matmul(ps2, lhsT=lh, rhs=yT[ci][0:hw, :],
                                     start=(ci == 0), stop=(ci == nhc - 1))
                nc.vector.tensor_add(out=xT[d], in0=xT[d], in1=ps2)
        # ---- final norm + tied lm head over vocab shard ----
        xf = rmsnorm()
        for d in range(8):
            dma(out=xf_in[d*128:(d+1)*128, :], in_=xf[d])
        nc.gpsimd.collective_compute(
            "AllGather", mybir.AluOpType.bypass, replica_groups=RG_ALL,
            ins=[xf_in.ap()], outs=[xf_out.ap()])
        xft = [pers.tile([128, 8, 256], F16, tag=f"xft{r}", name=f"xft{r}") for r in range(8)]
        for r in range(8):
            issue = nc.scalar.dma_start if r % 2 == 0 else dma
            issue(out=xft[r], in_=xf_out[r*D:(r+1)*D, :].rearrange("(dc p) t -> p dc t", p=128))
        for vc in range(8):
            embt = wp.tile([128, 8, 500], F16, tag="embt", name="embt", bufs=2)
            dma(out=embt, in_=P["embT"][:, vc*500:(vc+1)*500].rearrange("(dc p) v -> p dc v", p=128))
            for tcn in range(16):
                r, sl = divmod(tcn, 2)
                psl = ppb.tile([128, 512], F32, tag="pb", name="psl")
                for d in range(8):
                    nc.tensor.matmul(psl[:, 0:500], lhsT=xft[r][:, d, sl*128:(sl+1)*128],
                                     rhs=embt[:, d, :], start=(d == 0), stop=(d == 7))
                lg = act.tile([128, 500], F16, tag="lg", name="lg")
                if tcn % 2 == 0:
                    nc.vector.tensor_copy(out=lg, in_=psl[:, 0:500])
                else:
                    nc.scalar.activation(out=lg, in_=psl[:, 0:500], func=AF.Copy)
                (dma if tcn % 2 == 0 else gdma)(out=logits[tcn*128:(tcn+1)*128, vc*500:(vc+1)*500], in_=lg)
    nc.compile()
    return nc

_NC_CACHE = {}
def _get_nc():
    if "nc" not in _NC_CACHE:
        _NC_CACHE["nc"] = _build_nc()
    return _NC_CACHE["nc"]

def kernel(**inputs):
    from concourse.bass_utils import run_bass_kernel_spmd
    nc = _get_nc()
    in_maps = _host_prep(**inputs)
    res = run_bass_kernel_spmd(nc, in_maps, list(range(NCORES)))
    return _assemble(res.results)
